# revision 15
# baseline (speedup 1.0000x reference)
"""EmbeddingBag-mean (padded ragged gather + masked mean) on 8 Trainium2 cores.

Strategy (data-parallel over batch, per the sharding hint):
  - Each core owns 2048 samples (one sample per (block, partition) slot;
    16 blocks of 128). The embedding table is replicated to every core's
    HBM as bf16 rows padded to a 256B stride (dma_gather requires a 256B
    stride multiple; the 128B payload halves HBM gather traffic vs fp32,
    and bf16 stays ~100x inside the 2e-2 tolerance).
  - The gather uses the `dma_gather` extended instruction (streams
    thousands of rows per instruction; SWDGE descriptor generation on 4
    parallel Q7 queue pairs). Its indices are int16, so the vocabulary is
    split into 4 ranges of 32767 rows; each range gets a private zero row
    (local index 0) used to pad ragged per-sample slot counts. Per block
    and range, every sample's range-q indices occupy slots [0, n[b][q]);
    shorter samples point the tail slots at the range's zero row.
  - Host packing minimizes padding: samples are length-sorted into 128
    global blocks, blocks are profile-sorted and dealt 8-at-a-time to the
    same block position across cores, so the shared (one compiled module)
    per-(block, range) slot schedule hugs each block's true maxima.
  - Device, per block: <=4 dma_gathers into one [128, S_b, 64] bf16 tile,
    DVE tensor_reduce over slots (bf16 in, fp32 out), ACT copy-with-scale
    by 1/max(len,1) into a persistent [128, 16, 64] fp32 tile; one output
    DMA per iteration.
  - Host un-permutes (inverse of the global sort) to the original order.

The slot schedule depends on the input lengths, so the Bass module is
built per distinct schedule (cached).
"""

import numpy as np
import ml_dtypes

try:
    import concourse.bacc as bacc
except ImportError:  # harness containers keep the repo at /opt/trn_rl_repo
    import sys

    sys.path.insert(0, "/opt/trn_rl_repo")
    import concourse.bacc as bacc

import concourse.bass as bass
import concourse.mybir as mybir
import concourse.tile as tile
from concourse import bass_utils

B, L, V, D = 16384, 50, 100000, 64
NCORES = 8
P = 128
BC = B // NCORES  # 2048 samples per core
NBLK = BC // P  # 16 blocks of 128 samples
NRANGE = 4
RSTRIDE = 32768  # device rows per range (int16 local index 0..32767)
# vocab rows per range; the rest of each range is zero rows (padding targets,
# striped so padding gathers spread across HBM instead of hammering one row).
# Range 3 is deliberately tiny: fewer samples touch it, so its per-block max
# (and hence padding) stays small.
RVOCAB = 32000  # ranges 0-2
RVOCAB3 = V - 3 * RVOCAB  # 4000
DEVROWS = NRANGE * RSTRIDE

_CACHE: dict = {}


def _dma_gather_raw(gp, out_ap, in_ap, idxs_ap, num_idxs, elem_size, elem_step,
                    queue_num=0, single_packet=False):
    """bass.dma_gather minus the elem_size_bytes%256 assert (the firmware
    handles any elem size; only the row stride must be a 256B multiple).
    single_packet coalesces the CME stream into one SDMA packet -- only legal
    when descriptors/engine <= 64 (larger coalesced packets hang the SDMA)."""
    stride_bytes = elem_step * mybir.dt.size(in_ap.dtype)
    assert stride_bytes % 256 == 0 and stride_bytes // 256 < 256
    assert not single_packet or (num_idxs // 16 + 1) <= 64
    _in_ap = gp.lower_ap_dma(in_ap, for_custom_bir_dma=True)
    _idxs_ap = gp.lower_ap(idxs_ap)
    _out_ap = gp.lower_ap(out_ap)
    return gp.add_instruction(
        mybir.InstDMAGatherAnt(
            name=gp.bass.get_next_instruction_name(),
            ins=[*_in_ap, _idxs_ap, gp.lower_val_access(gp.to_reg(num_idxs))],
            outs=[_out_ap],
            transpose=False,
            num_idxs=num_idxs,
            elem_size=elem_size,
            stride_bytes_256=stride_bytes // 256,
            gen_mode=0,
            single_packet=single_packet,
            queue_num=queue_num,
            sbuf_tokens_per_rank=0,
            sbuf_free_dim_per_rank=0,
            sbuf_free_dim_pad_per_rank=0,
            sbuf_byte_offset=0,
        )
    )


def build(schedule, reps: int = 1, gbufs: int = 3):
    """Build + bacc-compile the per-core Bass module.

    schedule: NBLK x NRANGE tuple of per-(block, range) slot counts.
    reps > 1 wraps the block loop in tc.For_i (same outputs each
    iteration) -- used only for wall-clock slope timing in test.py.
    """
    sched = [list(r) for r in schedule]
    assert len(sched) == NBLK and all(len(r) == NRANGE for r in sched)
    s_blk = [sum(r) for r in sched]
    assert all(s > 0 for s in s_blk)
    s_max = max(s_blk)
    # free-dim int16 offsets of each (b, q) segment in the idx tensor
    offs, o = [], 0
    for b in range(NBLK):
        row = []
        for q in range(NRANGE):
            row.append(o)
            o += 8 * sched[b][q]
        offs.append(row)
    idx_w = o  # total int16 elements per partition

    nc = bacc.Bacc("TRN2", target_bir_lowering=False, debug=False,
                   num_swdge_queues=4)
    table = nc.dram_tensor("table", [DEVROWS, 2 * D], mybir.dt.bfloat16,
                           kind="ExternalInput")
    idx = nc.dram_tensor("idx", [P, idx_w], mybir.dt.int16, kind="ExternalInput")
    inv_len = nc.dram_tensor("inv_len", [P, NBLK], mybir.dt.float32,
                             kind="ExternalInput")
    out = nc.dram_tensor("out", [P, NBLK * D], mybir.dt.float32,
                         kind="ExternalOutput")

    with tile.TileContext(nc) as tc:
        with (
            tc.tile_pool(name="const", bufs=1) as cpool,
            tc.tile_pool(name="gather", bufs=gbufs) as gpool,
            tc.tile_pool(name="res", bufs=4) as rpool,
            tc.tile_pool(name="outp", bufs=2) as opool,
        ):
            idx_sb = cpool.tile([P, idx_w], mybir.dt.int16)
            nc.sync.dma_start(idx_sb[:], idx.ap())
            invl_sb = cpool.tile([P, NBLK], mybir.dt.float32)
            nc.sync.dma_start(invl_sb[:], inv_len.ap())

            def body():
                # NOTE: queue_num must follow the emission order round-robin:
                # the tile scheduler locks each DMASW sem lane to one SWDGE
                # queue, so any size-aware (LPT) queue assignment that breaks
                # the periodic instruction-order <-> queue pattern is rejected.
                outsb = opool.tile([P, NBLK, D], mybir.dt.float32, tag="outsb")
                qrr = 0
                for b in range(NBLK):
                    g = gpool.tile([P, s_max, D], mybir.dt.bfloat16, tag="g")
                    s = 0
                    for q in range(NRANGE):
                        nq = sched[b][q]
                        if nq == 0:
                            continue
                        _dma_gather_raw(
                            nc.gpsimd,
                            g[:, s : s + nq, :],
                            table.ap()[q * RSTRIDE : (q + 1) * RSTRIDE, 0:D],
                            idx_sb[:, offs[b][q] : offs[b][q] + 8 * nq],
                            128 * nq,
                            D,
                            2 * D,
                            queue_num=qrr % 4,
                            single_packet=(128 * nq // 16 + 1) <= 64,
                        )
                        qrr += 1
                        s += nq
                    red = rpool.tile([P, D], mybir.dt.float32, tag="red")
                    nc.vector.tensor_reduce(
                        out=red[:],
                        in_=g[:, : s_blk[b], :].rearrange("p l d -> p d l"),
                        axis=mybir.AxisListType.X,
                        op=mybir.AluOpType.add,
                    )
                    nc.scalar.activation(
                        outsb[:, b, :],
                        red[:],
                        mybir.ActivationFunctionType.Copy,
                        scale=invl_sb[:, b : b + 1],
                    )
                nc.sync.dma_start(out.ap(), outsb[:].rearrange("p b d -> p (b d)"))

            if reps == 1:
                body()
            else:
                with tc.For_i(0, reps, 1):
                    body()

    nc.compile()
    return nc


def preprocess(table, indices, lengths):
    """Host prep. Returns (in_maps, schedule, perms).

    perms[c][k] = original global sample id of device row k on core c
    (k = b*128 + p); the caller scatters core outputs back with it."""
    table = np.ascontiguousarray(np.asarray(table, dtype=np.float32))
    # device table: 4 ranges x (25000 vocab rows + 7768 zero rows), bf16 rows
    # padded to a 256B stride (payload in cols 0:64)
    tdev = np.zeros((DEVROWS, 2 * D), ml_dtypes.bfloat16)
    tb = table.astype(ml_dtypes.bfloat16)
    for q in range(NRANGE):
        lo, hi = q * RVOCAB, min((q + 1) * RVOCAB, V)
        if lo < hi:
            tdev[q * RSTRIDE : q * RSTRIDE + (hi - lo), :D] = tb[lo:hi]

    idx_all = np.asarray(indices).astype(np.int64)  # [B, L]
    lens = np.asarray(lengths).astype(np.int64)  # [B]
    valid = np.arange(L, dtype=np.int64)[None, :] < lens[:, None]
    rng = np.minimum(idx_all // RVOCAB, NRANGE - 1)  # range id per index
    loc = (idx_all - rng * RVOCAB).astype(np.int16)  # local idx within range
    # per-sample per-range counts
    cnt = np.zeros((B, NRANGE), np.int64)
    for q in range(NRANGE):
        cnt[:, q] = ((rng == q) & valid).sum(axis=1)

    # ---- global packing: k-d profile clustering, profile-dealt to cores ----
    def kd3(ids):
        if len(ids) <= P:
            return [ids]
        c = cnt[ids][:, :3]
        ax = int(np.argmax(c.max(0) - c.min(0)))
        o = ids[np.argsort(c[:, ax], kind="stable")]
        h = len(ids) // 2
        return kd3(o[:h]) + kd3(o[h:])

    order = np.concatenate(kd3(np.arange(B)))
    blocks = order.reshape(NCORES * NBLK, P)  # [128 blocks, 128 samples]
    bprof = cnt[blocks].max(axis=1)  # [128, NRANGE] per-block max
    # greedy profile-matched grouping: the shared schedule row is the
    # elementwise max over each group of 8 blocks, so group blocks with
    # near-identical profiles (beats sum-sorted dealing by ~5%)
    rem = list(np.argsort(-bprof.sum(axis=1), kind="stable"))
    groups = []
    while rem:
        seed = rem.pop(0)
        d = sorted((int(np.abs(bprof[r] - bprof[seed]).sum()), r) for r in rem)
        grp = [seed] + [r for _, r in d[: NCORES - 1]]
        for _, r in d[: NCORES - 1]:
            rem.remove(r)
        groups.append(grp)
    schedule = []
    perms = [np.empty(BC, np.int64) for _ in range(NCORES)]
    core_blocks = [[None] * NBLK for _ in range(NCORES)]
    for b in range(NBLK):
        grp = groups[b]
        row = bprof[grp].max(axis=0)
        if row.sum() == 0:
            row[0] = 1
        schedule.append(tuple(int(x) for x in row))
        for c in range(NCORES):
            core_blocks[c][b] = blocks[grp[c]]
            perms[c][b * P : (b + 1) * P] = blocks[grp[c]]
    schedule = tuple(schedule)

    inv_len = (1.0 / np.maximum(lens, 1)).astype(np.float32)
    idx_w = sum(8 * n for row in schedule for n in row)

    in_maps = []
    for c in range(NCORES):
        segs = []
        invl_dev = np.empty((P, NBLK), np.float32)
        for b in range(NBLK):
            sb = core_blocks[c][b]  # 128 global sample ids
            invl_dev[:, b] = inv_len[sb]
            for q in range(NRANGE):
                nq = schedule[b][q]
                if nq == 0:
                    continue
                # padding -> striped zero rows (distinct HBM addresses; a
                # single shared zero row serializes the SDMA engines on one
                # HBM bank and is ~5x slower end-to-end)
                zbase = RVOCAB3 if q == NRANGE - 1 else RVOCAB
                nzero = RSTRIDE - zbase
                flat = (
                    zbase
                    + (
                        np.arange(nq)[:, None] * 131
                        + np.arange(P)[None, :] * 61
                        + b * 17
                        + q * 7
                    )
                    % nzero
                ).astype(np.int16)  # [slot j, partition p]
                for p in range(P):
                    s = sb[p]
                    k = cnt[s, q]
                    if k:
                        flat[:k, p] = loc[s][valid[s] & (rng[s] == q)]
                fv = flat.ravel()  # idx_flat[j*128+p]
                seg = fv.reshape(-1, 16).T  # [16, n*8]
                segs.append(np.tile(seg, (8, 1)))
        idx_dev = np.concatenate(segs, axis=1)
        assert idx_dev.shape == (P, idx_w)
        in_maps.append(
            {
                "table": tdev,
                "idx": np.ascontiguousarray(idx_dev),
                "inv_len": np.ascontiguousarray(invl_dev),
            }
        )
    return in_maps, schedule, perms


def kernel(table, indices, lengths):
    in_maps, schedule, perms = preprocess(table, indices, lengths)
    nc = _CACHE.get(schedule)
    if nc is None:
        nc = _CACHE[schedule] = build(schedule)
    res = bass_utils.run_bass_kernel_spmd(nc, in_maps, core_ids=list(range(NCORES)))
    full = np.empty((B, D), np.float32)
    for c in range(NCORES):
        rows = res.results[c]["out"].reshape(P, NBLK, D)
        rows = np.ascontiguousarray(rows.transpose(1, 0, 2)).reshape(BC, D)
        full[perms[c]] = rows
    return full
